# revision 28
# baseline (speedup 1.0000x reference)
"""Per-image piecewise-linear LUT (histogram binning) kernel for Trainium2.

Strategy (pure data-parallel over 8 NeuronCores, batch sharded 2 per core):
- Host precomputes, per (b, c), a dense 256-entry nearest-neighbor table of
  the normalized curve at bin centers tbl[j] = round(255 * y(j/S)), S = 255.
  Dense sampling removes the on-device interpolation entirely; with u8 table
  values the end-to-end error is ~3.0e-3 norm-rel, far inside the 2e-2 gate.
- x ships as fp16 (halves input HBM traffic; fp16 quantization of x only
  perturbs the bin index by <0.15 bins).  Output is u8 codes, dequantized
  (/255) on host.
- On-device per core: 6 images of [128 partitions x 8192 fp16].  Per image:
    u16 idx = u16(255*x) + 256*(img&1)
        (DVE tensor_scalar mult+add; split 8191+1 so the mode detector picks
         2x_1P, which uses only the DVE's dedicated SBUF port - the 2-port 4x
         mode would lock the port pair shared with the pool engine and stall
         the gathers.  The +256 offset selects the image's half of the pool
         buffer, so one PoolBufferLoad serves TWO images: 3 PBLs of 512
         entries instead of 6.)
    out u8  = pooltable[idx]
        (pool-engine Gather; the stock Q7 gather ucode runs ~4.5 cyc/elem
         with random indices and is the kernel's bottleneck at ~190us/core)
    DMA out (u8, on the ACT HWDGE ring so its semaphore wait cannot block
     input DMAs queued on the SP ring)
- The raw Gather/PoolBufferLoad ISA instructions cannot carry semaphores
  (walrus rejects sync on unknown structs); drains bracket them and all
  cross-engine syncs land on the drains / are wired manually.
- Measured (loop-slope, 8 cores): 206.5us vs 371us staged baseline (1.80x).
  Ablations: DMA-only 82us, pool-gather-only ~191-196us (random indices) -
  the stock Q7 gather ucode, not HBM, is the binding constraint.
"""
import sys

sys.path.insert(0, "/opt/trn_rl_repo")

import numpy as np

B, C, H, W = 16, 3, 1024, 1024
K = 64
NCORES = 8
BPC = B // NCORES  # batches per core
IMGS = BPC * C  # images per core
P = 128
FREE = H * W // P  # 8192
CHUNK = 8192
NCHUNK = FREE // CHUNK
TBL = 256  # table entries per image (two tables share the 512-entry pool buffer)
S = 255.0  # index scale: u = round_nearest(x*S) in [0, TBL-1] for x in [0, 1]
NB = 3  # buffer depth

_cached = {}


def _build(loop_n=None, mode="full"):
    import contextlib
    import concourse.mybir as mybir
    from concourse.bacc import Bacc
    from concourse.tile import TileContext
    from concourse.tile_rust import add_dep_helper
    import concourse.bass_interp as _bi

    # Tile's scheduling simulator doesn't know these opcodes; no-op them there.
    _orig_visit = _bi._visit_InstISA

    def _patched_visit(isa, instruction, core_sim):
        if instruction.isa_opcode in (
            isa.Opcode.NEURON_ISA_TPB_OPCODE_POOL_BUFFER_LOAD.value,
            isa.Opcode.NEURON_ISA_TPB_OPCODE_GATHER.value,
        ):
            return
        return _orig_visit(isa, instruction, core_sim)

    _bi._visit_InstISA = _patched_visit

    nc = Bacc()
    dt = nc.isa.get_enum("NEURON_ISA_TPB_DTYPE")
    Op = nc.isa.Opcode
    ALU = mybir.AluOpType

    xs_d = nc.dram_tensor("xs", [IMGS, H, W], mybir.dt.float16, kind="ExternalInput")
    tb_d = nc.dram_tensor("tb", [IMGS, P, TBL], mybir.dt.uint8, kind="ExternalInput")
    os_d = nc.dram_tensor("os", [IMGS, H, W], mybir.dt.uint8, kind="ExternalOutput")

    xs_r = xs_d[:].rearrange("i (p r) c -> i p (r c)", p=P)
    os_r = os_d[:].rearrange("i (p r) c -> i p (r c)", p=P)

    with (
        nc.sbuf_tensor("tbl_all", [P, IMGS * TBL], mybir.dt.uint8) as tbl_all,
        nc.sbuf_tensor("tbl_cp", [P, IMGS * TBL], mybir.dt.uint8) as tbl_cp,
        nc.sbuf_tensor("xb", [P, NB * CHUNK], mybir.dt.float16) as xb,
        nc.sbuf_tensor("ub", [P, IMGS * CHUNK], mybir.dt.uint16) as ub,
        nc.sbuf_tensor("ob", [P, IMGS * CHUNK], mybir.dt.uint8) as ob,
        TileContext(nc) as tc,
    ):
        ub_off, _ = nc.gpsimd._ap_to_byte_offset(ub[:])
        xb_off, _ = nc.gpsimd._ap_to_byte_offset(xb[:])
        ob_off, _ = nc.gpsimd._ap_to_byte_offset(ob[:])
        tcp_off, _ = nc.gpsimd._ap_to_byte_offset(tbl_cp[:])
        U16 = dt.NEURON_ISA_TPB_DTYPE_UINT16.value
        F16 = dt.NEURON_ISA_TPB_DTYPE_FP16.value
        U8 = dt.NEURON_ISA_TPB_DTYPE_UINT8.value

        loop_cm = (
            tc.For_i(0, loop_n, 1) if loop_n is not None else contextlib.nullcontext()
        )
        if mode == "dma":
            with loop_cm:
                for img in range(IMGS):
                    for cidx in range(NCHUNK):
                        k = img * NCHUNK + cidx
                        slot = k % NB
                        f0 = cidx * CHUNK
                        so = slot * CHUNK
                        nc.sync.dma_start(
                            xb[:, so : so + CHUNK], xs_r[img, :, f0 : f0 + CHUNK]
                        )
                        nc.scalar.dma_start(
                            os_r[img, :, f0 : f0 + CHUNK], ob[:, so : so + CHUNK]
                        )
        elif mode == "pool":
            DT_MAP = {
                "u8": (dt.NEURON_ISA_TPB_DTYPE_UINT8.value, 1),
                "u16": (U16, 2),
                "u32": (dt.NEURON_ISA_TPB_DTYPE_UINT32.value, 4),
                "f16": (F16, 2),
            }
            idx_e, idx_b = DT_MAP[P_IDX]
            out_e, out_b = DT_MAP[P_OUT]
            tbl_n = P_TBL_N
            rmp_d = nc.dram_tensor(
                "rmp", [P, CHUNK * idx_b], mybir.dt.uint8, kind="ExternalInput"
            )
            with loop_cm:
                for img in range(IMGS):
                    nc.sync.dma_start(
                        tbl_all[:, img * TBL : (img + 1) * TBL], tb_d[img]
                    )
                tbl_touch = nc.vector.tensor_copy(tbl_cp[:], tbl_all[:])
                nc.sync.dma_start(
                    xb[:].bitcast(mybir.dt.uint8)[:, : CHUNK * idx_b], rmp_d[:]
                )
                zed = nc.vector.tensor_copy(
                    ub[:, : CHUNK * idx_b // 2],
                    xb[:].bitcast(mybir.dt.uint16)[:, : CHUNK * idx_b // 2],
                )
                prev_pool = None
                for img in range(IMGS):
                    for cidx in range(NCHUNK):
                        k = img * NCHUNK + cidx
                        so = (k % NB) * CHUNK
                        pre = nc.gpsimd.drain()
                        if prev_pool is not None:
                            add_dep_helper(pre.ins, prev_pool.ins, sync=False,
                                           reason="pool order")
                        if k == 0:
                            add_dep_helper(pre.ins, tbl_touch.ins, sync=True,
                                           reason="tables")
                            add_dep_helper(pre.ins, zed.ins, sync=True,
                                           reason="idx loaded")
                        if cidx == 0:
                            pbl = nc.gpsimd.isa(
                                Op.NEURON_ISA_TPB_OPCODE_POOL_BUFFER_LOAD,
                                {
                                    "src_mem_pattern": {
                                        "start_addr": {
                                            "addr_immediate": int(tcp_off)
                                            + img * TBL * 2
                                        },
                                        "num_elem": [tbl_n, 1, 1, 1],
                                        "step_elem": [1, 0, 0, 0],
                                    },
                                    "in_dtype": out_e,
                                    "num_active_channels": P,
                                    "start_index": 0,
                                    "mask": tbl_n - 1,
                                },
                            )
                            add_dep_helper(pbl.ins, pre.ins, sync=False,
                                           reason="pool order")
                            gdep = pbl
                        else:
                            gdep = pre
                        gt = nc.gpsimd.isa(
                            Op.NEURON_ISA_TPB_OPCODE_GATHER,
                            {
                                "src_mem_pattern": {
                                    "start_addr": {"addr_immediate": int(ub_off)},
                                    "num_elem": [CHUNK, 1, 1, 1],
                                    "step_elem": [1, 0, 0, 0],
                                },
                                "dst_mem_pattern": {
                                    "start_addr": {"addr_immediate": int(ob_off)},
                                    "num_elem": [CHUNK, 1, 1, 1],
                                    "step_elem": [1, 0, 0, 0],
                                },
                                "in_dtype": idx_e,
                                "out_dtype": out_e,
                                "num_active_channels": P,
                                "index_miss_behavior": 0,
                                "immediate": {"imm_bitvec_uint32": 0},
                                "free_pool_buffer": 0,
                            },
                        )
                        add_dep_helper(gt.ins, gdep.ins, sync=False,
                                       reason="pool order")
                        prev_pool = gt
                fin = nc.gpsimd.drain()
                add_dep_helper(fin.ins, prev_pool.ins, sync=False,
                               reason="pool order")
        if mode in ("dma", "pool"):
            pass
        else:
          with loop_cm:
            # table load + a DVE copy so pool's wait collapses onto the DVE clock
            for img in range(IMGS):
                nc.sync.dma_start(tbl_all[:, img * TBL : (img + 1) * TBL], tb_d[img])
            tbl_touch = nc.vector.tensor_copy(tbl_cp[:], tbl_all[:])

            fences = {}  # k -> drain emitted just after gather k-1 (pool order)
            outs = {}  # k -> output DMA instruction for chunk k
            pend = None  # (k, img, f0, slot) awaiting its post-gather fence
            prev_pool = None
            k = 0

            def _emit_out(p, fence):
                d = nc.scalar.dma_start(
                    os_r[p["img"], :, p["f0"] : p["f0"] + CHUNK],
                    ob[:, p["slot"] * CHUNK : (p["slot"] + 1) * CHUNK],
                )
                add_dep_helper(d.ins, fence.ins, sync=True, reason="gather done")
                outs[p["k"]] = d

            for img in range(IMGS):
                for cidx in range(NCHUNK):
                    slot = k % NB
                    f0 = cidx * CHUNK
                    so = slot * CHUNK
                    x_t = xb[:, so : so + CHUNK]
                    u_t = ub[:, so : so + CHUNK]

                    nc.sync.dma_start(x_t, xs_r[img, :, f0 : f0 + CHUNK])

                    # idx = u16(S*x), round-nearest: bin centers at j/S.
                    # Odd element count forces DVE mode 2x_1P (single dedicated
                    # read port) instead of 4x_2P - the 2-port modes grab the
                    # SBUF port pair shared with the pool engine and would
                    # stall the gathers.
                    toff = float(TBL * (img & 1))
                    ts_u = nc.vector.tensor_scalar(
                        ub[:, so : so + CHUNK - 1], xb[:, so : so + CHUNK - 1],
                        float(S), toff, ALU.mult, ALU.add,
                    )
                    ts_u2 = nc.vector.tensor_scalar(
                        ub[:, so + CHUNK - 1 : so + CHUNK],
                        xb[:, so + CHUNK - 1 : so + CHUNK],
                        float(S), toff, ALU.mult, ALU.add,
                    )
                    add_dep_helper(ts_u2.ins, ts_u.ins, sync=False, reason="dve order")
                    if k >= NB:
                        # gather k-NB read this ub slot; its fence is fences[k-NB+1]
                        # (ts_u2 is queue-ordered after ts_u, so one dep suffices)
                        add_dep_helper(
                            ts_u.ins, fences[k - NB + 1].ins, sync=True,
                            reason="u WAR",
                        )

                    # pool: single drain per chunk — serves as the previous
                    # gather's completion fence AND this gather's input wait
                    pre = nc.gpsimd.drain()
                    fences[k] = pre
                    if prev_pool is not None:
                        add_dep_helper(
                            pre.ins, prev_pool.ins, sync=False, reason="pool order"
                        )
                    add_dep_helper(pre.ins, ts_u2.ins, sync=True, reason="u ready")
                    if k >= NB:
                        # out-DMA k-NB still reads this ob slot
                        add_dep_helper(
                            pre.ins, outs[k - NB].ins, sync=True, reason="o WAR"
                        )
                    if cidx == 0 and img % 2 == 0:
                        if img == 0:
                            add_dep_helper(
                                pre.ins, tbl_touch.ins, sync=True, reason="tables"
                            )
                        pbl = nc.gpsimd.isa(
                            Op.NEURON_ISA_TPB_OPCODE_POOL_BUFFER_LOAD,
                            {
                                "src_mem_pattern": {
                                    "start_addr": {
                                        "addr_immediate": int(tcp_off) + img * TBL
                                    },
                                    "num_elem": [2 * TBL, 1, 1, 1],
                                    "step_elem": [1, 0, 0, 0],
                                },
                                "in_dtype": U8,
                                "num_active_channels": P,
                                "start_index": 0,
                                "mask": 2 * TBL - 1,
                            },
                        )
                        add_dep_helper(pbl.ins, pre.ins, sync=False, reason="pool order")
                        gdep = pbl
                    else:
                        gdep = pre
                    gt = nc.gpsimd.isa(
                        Op.NEURON_ISA_TPB_OPCODE_GATHER,
                        {
                            "src_mem_pattern": {
                                "start_addr": {"addr_immediate": int(ub_off) + so * 2},
                                "num_elem": [CHUNK, 1, 1, 1],
                                "step_elem": [1, 0, 0, 0],
                            },
                            "dst_mem_pattern": {
                                "start_addr": {"addr_immediate": int(ob_off) + so},
                                "num_elem": [CHUNK, 1, 1, 1],
                                "step_elem": [1, 0, 0, 0],
                            },
                            "in_dtype": U16,
                            "out_dtype": U8,
                            "num_active_channels": P,
                            "index_miss_behavior": 0,
                            "immediate": {"imm_bitvec_uint32": 0},
                            "free_pool_buffer": 0,
                        },
                    )
                    add_dep_helper(gt.ins, gdep.ins, sync=False, reason="pool order")

                    # the drain just emitted fences the PREVIOUS gather; its
                    # output can ship now
                    if pend is not None:
                        _emit_out(pend, pre)
                    pend = dict(k=k, img=img, f0=f0, slot=slot)
                    prev_pool = gt
                    k += 1
            fin = nc.gpsimd.drain()
            add_dep_helper(fin.ins, prev_pool.ins, sync=False, reason="pool order")
            _emit_out(pend, fin)

    nc.finalize()
    return nc


def _tables(un_normalized_y: np.ndarray) -> np.ndarray:
    """[B, C, TBL] u8: dense midpoint-sampled LUT of the normalized curve,
    quantized to u8 (host dequantizes the gathered codes by /255)."""
    u = un_normalized_y.astype(np.float64)
    h = np.logaddexp(0.0, u)  # softplus
    y = np.cumsum(h, axis=2)
    y0 = y[:, :, :1]
    yn = y[:, :, -1:]
    y = (y - y0) / (yn - y0)  # [B, C, K+1], y[0]=0, y[K]=1

    t = np.minimum(np.arange(TBL, dtype=np.float64) / S, 1.0)  # bin centers
    scaled = t * K
    idx0 = np.clip(np.floor(scaled), 0, K - 1).astype(np.int64)
    alpha = scaled - idx0
    y_lo = y[:, :, idx0]  # [B, C, TBL]
    y_hi = y[:, :, idx0 + 1]
    val = y_lo + alpha * (y_hi - y_lo)
    return np.round(val * 255.0).astype(np.uint8)


RMP_KIND = "ramp"  # bench-only: index pattern for the pool ablation
P_IDX = "u16"  # bench-only: pool ablation gather index dtype
P_OUT = "f16"  # bench-only: pool ablation gather data dtype
P_TBL_N = TBL  # bench-only: pool ablation table entries


def _in_maps(x: np.ndarray, uy: np.ndarray, with_rmp=False):
    pk = _tables(uy)
    x16 = x.astype(np.float16)
    rmp = None
    if with_rmp:  # bench-only: host-supplied index pattern for the pool ablation
        np_idx = {"u8": np.uint8, "u16": np.uint16, "u32": np.uint32}[P_IDX]
        if RMP_KIND == "rand":
            rmp = np.random.default_rng(0).integers(
                0, P_TBL_N, size=(P, CHUNK), dtype=np_idx
            )
        else:
            rmp = (np.broadcast_to(
                (np.arange(CHUNK) % P_TBL_N)[None, :], (P, CHUNK)
            )).astype(np_idx)
        rmp = np.ascontiguousarray(rmp).view(np.uint8).reshape(P, -1)
    in_maps = []
    for c in range(NCORES):
        xs = x16[c * BPC : (c + 1) * BPC].reshape(IMGS, H, W)
        tb = np.ascontiguousarray(
            np.broadcast_to(
                pk[c * BPC : (c + 1) * BPC].reshape(IMGS, 1, TBL), (IMGS, P, TBL)
            )
        )
        m = {"xs": np.ascontiguousarray(xs), "tb": tb}
        if rmp is not None:
            m["rmp"] = rmp
        in_maps.append(m)
    return in_maps


def kernel(x: np.ndarray, un_normalized_y: np.ndarray) -> np.ndarray:
    from concourse import bass_utils

    x = np.asarray(x, dtype=np.float32)
    uy = np.asarray(un_normalized_y, dtype=np.float32)

    if "nc" not in _cached:
        _cached["nc"] = _build()
    nc = _cached["nc"]

    res = bass_utils.run_bass_kernel_spmd(
        nc, _in_maps(x, uy), core_ids=list(range(NCORES))
    )
    out = np.empty((B, C, H, W), dtype=np.float32)
    inv255 = np.float32(1.0 / 255.0)
    for c in range(NCORES):
        out[c * BPC : (c + 1) * BPC] = (
            res.results[c]["os"].astype(np.float32).reshape(BPC, C, H, W) * inv255
        )
    return out
